# revision 7
# baseline (speedup 1.0000x reference)
"""KNN-classifier kernel for Trainium2 (8 NeuronCores, SPMD).

Strategy:
  - Shard train_features row-wise across 8 cores (12500 rows each),
    single launch per core; q resident in SBUF, t-shard streamed in 5
    double-buffered chunks.
  - sim = features_rank @ shard.T in ONE fp8 e4m3 pass with
    perf_mode=DoubleRow (256-deep contraction per matmul, ~1.6x the
    fp16 PE rate; fp32 PSUM accumulation).
  - Per 500-column tile: PSUM bounced to SBUF as bf16 by the otherwise
    idle Scalar engine, then DVE top-8 values + indices (max/max_index)
    -> 200 candidates per row per core.
  - Host: merge the 8x200 candidates, take the top-200 by approx value,
    exactly rescore (fp32 dot) the top-96 of every row -- the fp8 sims
    are ~+-7 off, but at T=0.07 the softmax weights of everything
    outside the true top handful underflow to exactly 0, and the true
    top candidates sit far inside the approx top-96 -- then softmax +
    weighted class histograms exactly mirroring the reference math.

Measured: 539us HW exec (one NTFF-profiled launch, 8 cores), rel_err
5.3e-4 vs the fp32 reference (gate 2e-2). The baseline 3-pass fp16
version ran 2.39ms; the 1-pass fp16 variant 733us (PE-roofline-bound);
this version is DVE-bound (top-8 scans), PE 68% busy.
"""

import sys

sys.path.insert(0, "/opt/trn_rl_repo")

import numpy as np

B = 2048
D = 1024
NTRAIN = 100000
NCORES = 8
NLOC = NTRAIN // NCORES    # 12500
TS = 500
NT = NLOC // TS            # 25
GT = 5
NG = NT // GT              # 5
KC = D // 128              # 8 x 128 contraction chunks (4 DoubleRow pairs)
KP = KC // 2               # 4 pairs
BT = B // 128
CPT = NT * 8               # 200
TPAD = GT * TS + 12        # 2512: k-dim stride %16==0 for DoubleRow APs
SB_BUFS = 8
MAXK = 200
TEMP = 0.07
NB_KNN = (10, 20, 100, 200)
NUM_CLASSES = 1000
RESCORE_POOL = 96

_CACHE = {}


def _build():
    from concourse import bass, tile, mybir

    if not getattr(tile.TileContext, "_drain_split_patched", False):
        from concourse.vector_clock import ScopedClock

        def _split_drain(self, tick_clock, wait_clock):
            drain_inst = self.nc.sync.drain()
            wait_clock.add_sem_waits(
                drain_inst.ins, ScopedClock({None: tick_clock.global_clock})
            )
            si = drain_inst.ins.sync_info
            if si is not None and si.on_wait and len(si.on_wait) > 1:
                waits = list(si.on_wait)
                try:
                    si.on_wait[:] = waits[:1]
                except Exception:
                    drain_inst.ins.sync_info = mybir.SyncInfo(
                        on_wait=waits[:1], on_update=list(si.on_update))
                for wt in waits[1:]:
                    d2 = self.nc.sync.drain()
                    s2 = d2.ins.sync_info
                    if s2 is None:
                        d2.ins.sync_info = mybir.SyncInfo(
                            on_wait=[wt], on_update=[])
                    else:
                        try:
                            s2.on_wait[:] = [wt]
                        except Exception:
                            d2.ins.sync_info = mybir.SyncInfo(
                                on_wait=[wt], on_update=list(s2.on_update))
            self.nc.all_engine_barrier()
            popped = self.nc._tile_sem_poison_stack.pop()
            assert popped is self._sem_poison
            self.nc.clear_and_free_semaphores(
                list(self.sems.allocated().values()))
            self.nc.all_engine_barrier()

        tile.TileContext._drain_and_barrier = _split_drain
        tile.TileContext._drain_split_patched = True

    F8 = mybir.dt.float8e4
    BF16 = mybir.dt.bfloat16
    F32 = mybir.dt.float32
    U32 = mybir.dt.uint32
    DR = mybir.MatmulPerfMode.DoubleRow

    nc = bass.Bass()
    qT = nc.declare_dram_parameter("qT", [D, B], F8, isOutput=False)
    tT = nc.declare_dram_parameter("tT", [D, NLOC], F8, isOutput=False)
    out_v = nc.declare_dram_parameter("out_v", [B, CPT], BF16, isOutput=True)
    out_i = nc.declare_dram_parameter("out_i", [B, CPT], U32, isOutput=True)

    qT3 = qT.rearrange("(k p) b -> p k b", p=128)
    tT3 = tT.rearrange("(k p) n -> p k n", p=128)
    ov3 = out_v.rearrange("(b p) c -> p b c", p=128)
    oi3 = out_i.rearrange("(b p) c -> p b c", p=128)

    with tile.TileContext(nc) as tc:
        with (
            tc.tile_pool(name="qpool", bufs=1) as qpool,
            tc.tile_pool(name="tpool", bufs=2) as tpool,
            tc.tile_pool(name="bpool", bufs=SB_BUFS) as bpool,
            tc.tile_pool(name="spool", bufs=1) as spool,
            tc.tile_pool(name="ppool", bufs=8, space="PSUM") as ppool,
        ):
            q8 = qpool.tile([128, KC, B], F8)
            nc.gpsimd.dma_start(out=q8[:], in_=qT3[:])
            vals16 = spool.tile([128, BT * CPT], BF16, name="vals16")
            idx32 = spool.tile([128, BT * CPT], U32, name="idx32")
            scr = spool.tile([128, 8], U32, name="scr")

            idx_hist = []        # idx32 slices, for ACT wait-absorbers
            for g in range(NG):
                t8 = tpool.tile([128, KC, TPAD], F8, tag="t8")
                nc.gpsimd.dma_start(
                    out=t8[:, :, :GT * TS],
                    in_=tT3[:, :, g * GT * TS:(g + 1) * GT * TS])
                # PE-queue wait-absorbers for the DMA completions (one
                # sync-wait max per TPB instruction).
                if g == 0:
                    nc.tensor.ldweights(weights=q8[:, 0, 0:128])
                nc.tensor.ldweights(weights=t8[:, 0, 0:128])
                for b in range(BT):
                    bs = slice(b * 128, (b + 1) * 128)
                    pss = [ppool.tile([128, TS], F32, tag="ps",
                                      name=f"ps_{g}_{b}_{i}")
                           for i in range(GT)]
                    for k2 in range(KP):
                        for ti in range(GT):
                            nc.tensor.matmul(
                                out=pss[ti][:],
                                lhsT=q8[:, 2 * k2:2 * k2 + 2, bs],
                                rhs=t8[:, 2 * k2:2 * k2 + 2,
                                       ti * TS:(ti + 1) * TS],
                                perf_mode=DR,
                                start=(k2 == 0), stop=(k2 == KP - 1),
                            )
                    for ti in range(GT):
                        tg = g * GT + ti
                        vsl = slice(b * CPT + tg * 8, b * CPT + tg * 8 + 8)
                        # ACT wait-absorber: the sb16 slot reused now was
                        # last read by the max_index 8 tiles ago; a tiny
                        # ACT copy of that max_index's output takes the
                        # DVE wait so the real copy only waits on PE.
                        if len(idx_hist) >= SB_BUFS:
                            nc.scalar.copy(
                                out=scr[:], in_=idx_hist[-SB_BUFS])
                        s16 = bpool.tile([128, TS], BF16, tag="s16",
                                         name=f"s16_{g}_{b}_{ti}")
                        nc.scalar.copy(out=s16[:], in_=pss[ti][:])
                        nc.vector.max(out=vals16[:, vsl], in_=s16[:])
                        nc.vector.max_index(
                            out=idx32[:, vsl], in_max=vals16[:, vsl],
                            in_values=s16[:])
                        idx_hist.append(idx32[:, vsl])
            nc.gpsimd.dma_start(out=ov3[:], in_=vals16[:])
            nc.gpsimd.dma_start(out=oi3[:], in_=idx32[:])

    # One sync-wait max per TPB/DMA instruction on the PJRT path. Two
    # classes of redundant waits are dropped:
    #  - same-engine waits: every TPB engine executes its queue in order,
    #    so a wait on the instruction's own completion sem is implied
    #  - DMASW waits on DMAs that also wait on PE: the PE readers being
    #    waited on already waited on that DMA's completion (WAW covered)
    own_sem = {"InstActivation": "Activation", "InstMax": "Vector",
               "InstMaxIndex": "Vector", "InstTensorScalarPtr": "Vector",
               "InstMatmult": "PE", "InstLdweights": "PE"}
    for blk in nc.m.functions[0].blocks:
        for ins in blk.instructions:
            si = getattr(ins, "sync_info", None)
            if si is None or not si.on_wait or len(si.on_wait) <= 1:
                continue
            waits = list(si.on_wait)
            pfx = own_sem.get(type(ins).__name__)
            if pfx is not None:
                waits = [w for w in waits if not w.ant_name.startswith(pfx)]
            if (type(ins).__name__ == "InstDMACopy"
                    and any(w.ant_name.startswith("PE") for w in waits)):
                waits = [w for w in waits
                         if not w.ant_name.startswith("DMASW")]
            assert len(waits) == 1, (ins, si.on_wait)
            try:
                si.on_wait[:] = waits
            except Exception:
                ins.sync_info = mybir.SyncInfo(
                    on_wait=waits, on_update=list(si.on_update))
    return nc


def _run_device(q, t, trace=False):
    from concourse.bass_utils import run_bass_kernel_spmd
    import ml_dtypes

    if "nc" not in _CACHE:
        _CACHE["nc"] = _build()
    nc = _CACHE["nc"]

    q8 = np.ascontiguousarray(q.astype(ml_dtypes.float8_e4m3).T)
    in_maps = []
    for c in range(NCORES):
        sh = t[c * NLOC:(c + 1) * NLOC].astype(ml_dtypes.float8_e4m3)
        in_maps.append({"qT": q8, "tT": np.ascontiguousarray(sh.T)})
    res = run_bass_kernel_spmd(nc, in_maps, core_ids=list(range(NCORES)),
                               trace=trace)
    if trace:
        _run_device.last_exec_ns = res.exec_time_ns

    tile_base = np.arange(NT, dtype=np.int64).repeat(8) * TS
    cvs, cis = [], []
    for c in range(NCORES):
        cvs.append(res.results[c]["out_v"].astype(np.float32))
        cis.append(res.results[c]["out_i"].astype(np.int64)
                   + (c * NLOC + tile_base))
    return np.concatenate(cvs, axis=1), np.concatenate(cis, axis=1)


def kernel(features_rank, train_features, train_labels):
    q = np.ascontiguousarray(np.asarray(features_rank), dtype=np.float32)
    t = np.ascontiguousarray(np.asarray(train_features), dtype=np.float32)
    lab = np.asarray(train_labels)

    cv, ci = _run_device(q, t)

    part = np.argpartition(-cv, MAXK - 1, axis=1)[:, :MAXK]
    pv = np.take_along_axis(cv, part, axis=1)
    pi = np.take_along_axis(ci, part, axis=1)
    order = np.lexsort((pi, -pv), axis=1)
    topv = np.take_along_axis(pv, order, axis=1)
    topi = np.take_along_axis(pi, order, axis=1)

    # fp8 sims are ~+-7 off: exactly rescore the top-RESCORE_POOL of every
    # row (the true top few are inside by a wide margin), re-sort, then
    # softmax. The tail (pool..200) keeps approx values; its true weights
    # underflow to exactly 0 at T=0.07 regardless.
    p2 = RESCORE_POOL
    sub_i = topi[:, :p2]
    ex = np.empty((B, p2), np.float32)
    CH = 256
    for i in range(0, B, CH):
        ex[i:i + CH] = (t[sub_i[i:i + CH]] @ q[i:i + CH, :, None])[:, :, 0]
    o2 = np.lexsort((sub_i, -ex), axis=1)
    topv[:, :p2] = np.take_along_axis(ex, o2, axis=1)
    topi[:, :p2] = np.take_along_axis(sub_i, o2, axis=1)

    x = topv / np.float32(TEMP)
    x -= x.max(axis=1, keepdims=True)
    e = np.exp(x, dtype=np.float32)
    w = e / e.sum(axis=1, keepdims=True, dtype=np.float32)

    nl = lab[topi].astype(np.int64)
    flat_base = np.arange(B, dtype=np.int64)[:, None] * NUM_CLASSES
    probas = []
    for k in NB_KNN:
        p = np.bincount((nl[:, :k] + flat_base).ravel(),
                        weights=w[:, :k].astype(np.float64).ravel(),
                        minlength=B * NUM_CLASSES)
        probas.append(p.reshape(B, NUM_CLASSES).astype(np.float32))
    return tuple(probas)
